# revision 23
# baseline (speedup 1.0000x reference)
"""Trainium2 Bass kernel for nn_Decoder (per-depth label classifier).

Math (per depth d with c_d labels, COUNTS=[16,128,512]):
    g_d = label_aware_embedding[:, idx_d, :].reshape(B, c_d*H)
    x_d = g_d @ W1_d.T                     # [B, H]
    logits_d = x_d @ Wp_d.T + bp_d         # [B, c_d]
    pred[:, idx_d] = logits_d

Sharding: the W1_d contraction dim (c_d*H) is split across 8 cores
(each core gets c_d/8 labels' worth of W1 columns plus the matching
gathered-embedding slice) and each core computes a partial x_d.
Because the predictor is linear in x, the cross-core reduction commutes
past it:  pred = (sum_i x_i) @ Wp.T = sum_i (x_i @ Wp.T).  So each core
runs the (tiny) predictor on its own partial x and the host unshard step
sums the 8 partial outputs and adds the bias once — no on-device
collective at all.

The kernel is DMA-bound on the W1 stream, so both W1 (x64) and the
gathered embedding (x2) are stored in HBM as fp8 e3m4 (Trainium's
4-mantissa-bit fp8), nearly halving HBM traffic vs bf16.  The x128
combined scale is folded into the host-packed predictor weights, so the
device tail is scale-free.  Measured rel err ~1.6e-2 vs the 2e-2
tolerance; inputs and HW accumulation are deterministic, so this margin
is stable run to run.

Device layout notes:
  - host pre-transposes so the contraction dim is the partition dim and
    every DMA reads a per-partition-contiguous span; W and g bytes are
    packed per 128-row K-chunk into one uint8 stream (576 B per chunk
    per partition: 512 B W.T e3m4, 64 B g.T e3m4) and the matmul APs
    bitcast the two sub-ranges back to e3m4.
  - main matmul: lhsT = gT chunk [128,64] (stationary), rhs = W chunk
    [128,512] (moving) -> psum [64,512] f32 per depth.
  - depths are processed largest-first (512, 128, 16 labels) so the big
    depth's tail (transpose + predictor + drain) hides inside the next
    depth's matmul stream; only the 16-label depth's tiny tail runs
    after the last main matmul.
  - the predictor needs x.T; partial x is cast to bf16 per 128-column
    block (so the PE transposes start after the first block, not the
    full cast) and transposed on the PE via identity matmuls.
"""

import sys

sys.path.insert(0, "/opt/trn_rl_repo")

import numpy as np
import ml_dtypes

import concourse.bass as bass
import concourse.bacc as bacc
import concourse.tile as tile
import concourse.mybir as mybir
from concourse import bass_utils

# bass_utils' trace path (taken when BASS_TRACE is set in the environment)
# imports antenv.axon_hooks, which this image's antenv package lacks.  Provide
# it: wire the real NTFF hook from trn_agent_boot when available, else a stub
# that degrades to an untraced run.  Also make the artifact upload a no-op
# (no bucket access here).
try:
    from antenv import axon_hooks as _axon_hooks  # noqa: F401
except ImportError:
    import types as _types

    def _make_hook():
        try:
            import trn_agent_boot.trn_boot as _tb

            return _tb._ntff_profile_via_ctypes("/opt/axon/libaxon_pjrt.so")
        except Exception:
            return None

    _hook = _make_hook()
    _mod = _types.ModuleType("antenv.axon_hooks")
    _mod.get_axon_ntff_profile_hook = lambda: _hook
    _mod.set_axon_ntff_profile_hook = lambda h: None
    sys.modules["antenv.axon_hooks"] = _mod
    bass_utils.upload_artifacts = lambda tmpdir: tmpdir

BF16 = np.dtype(ml_dtypes.bfloat16)
E3M4 = np.dtype(ml_dtypes.float8_e3m4)
WSCALE = 64.0  # W1 stored as e3m4 * WSCALE
GSCALE = 2.0  # gathered embedding stored as e3m4 * GSCALE

N_CORES = 8
H = 512
B = 64
COUNTS = [16, 128, 512]
L = sum(COUNTS)  # 656

# Fixed label->depth assignment (identical to the reference's module-level rng)
_depths = np.random.default_rng(0).permutation(np.repeat(np.arange(1, 4), COUNTS))
IDX = [np.where(_depths == d)[0] for d in (1, 2, 3)]
ORDER = np.concatenate(IDX)

PER_CORE = [c // N_CORES for c in COUNTS]  # labels per core per depth: [2, 16, 64]
KCH = [n * H // 128 for n in PER_CORE]  # K-chunks per depth per core: [8, 64, 256]
NCH = sum(KCH)  # 328

# bytes per K-chunk per partition: 512 B of W.T then 64 B of g.T (both e3m4)
WB = 512
GB = B
CHB = WB + GB  # 576

# processing order: largest depth first so its tail overlaps later streams
PROC = [2, 1, 0]
# DMA group sizes (in K-chunks) per processing position; small leading
# groups so the PE starts working as soon as possible, larger groups in
# steady state to cut per-transfer issue overhead.
GROUPS = {
    2: [4, 4, 8] + [8] * 4 + [16] * 13,  # 256 chunks
    1: [16] * 4,  # 64
    0: [4, 2, 2],  # 8
}
# chunk offset of each depth within the packed stream (processing order)
CHUNK_OFF = {2: 0, 1: KCH[2], 0: KCH[2] + KCH[1]}

LABEL_OFF = [0, COUNTS[0], COUNTS[0] + COUNTS[1]]  # predT row offset per depth

_CACHE = {}


def _build_module():
    f32 = mybir.dt.float32
    bf16 = mybir.dt.bfloat16
    e3m4 = mybir.dt.float8e3
    u8 = mybir.dt.uint8

    nc = bacc.Bacc("TRN2", target_bir_lowering=False, debug=False, num_devices=N_CORES)

    wg = nc.dram_tensor("wg", [128, NCH * CHB], u8, kind="ExternalInput").ap()
    wpt = nc.dram_tensor("wpt", [128, 4 * L], bf16, kind="ExternalInput").ap()
    ident = nc.dram_tensor("ident", [128, 128], bf16, kind="ExternalInput").ap()
    predT = nc.dram_tensor("predT", [L, B], f32, kind="ExternalOutput").ap()

    with tile.TileContext(nc) as tc:
        with (
            tc.tile_pool(name="wpool", bufs=12) as wpool,
            tc.tile_pool(name="consts", bufs=1) as consts,
            tc.tile_pool(name="xpool", bufs=1) as xpool,
            tc.tile_pool(name="spool", bufs=6) as spool,
            tc.tile_pool(name="ps_x", bufs=3, space="PSUM") as ps_x,
            tc.tile_pool(name="ps_t", bufs=2, space="PSUM") as ps_t,
            tc.tile_pool(name="ps_p", bufs=2, space="PSUM") as ps_p,
        ):
            # the predictor-weight load (672 KB) is deferred onto a wg HWDGE
            # ring mid-stream (below): its transfer then lands inside the
            # DMA slack opened up by the first depth's tail PE work, instead
            # of delaying the latency-critical leading wg groups.  The tiny
            # identity goes on gpsimd (SWDGE) right away.
            wpt_sb = consts.tile([128, 4 * L], bf16)
            id_sb = consts.tile([128, 128], bf16)
            nc.gpsimd.dma_start(id_sb[:], ident[:])

            # depth-d tail: cast partial x to bf16 per 128-col block,
            # transpose on the PE, then the partial predictor
            # logits_d.T = Wp_d @ x_d.T, drained to DRAM per 128-label
            # m-chunk.  For all but the last processed depth this is
            # emitted one group into the next depth's matmul stream, so
            # its PE work hides in DMA slack there.
            def emit_tail(d, ps, final=False):
                # the casts serialize regardless of engine (single PSUM-bank
                # read port), so keep them all on DVE — cross-engine hops
                # only add semaphore latency to the chain
                xbk = []
                for k in range(4):
                    xb = xpool.tile([B, 128], bf16, name=f"xb{d}_{k}", tag=f"xb{d}_{k}")
                    nc.vector.tensor_copy(xb[:], ps[:, k * 128 : (k + 1) * 128])
                    xbk.append(xb)
                pt = ps_t.tile([128, 4 * B], bf16, name=f"pt{d}", tag="pt")
                for k in range(4):
                    nc.tensor.transpose(
                        pt[:, k * B : (k + 1) * B], xbk[k][:], id_sb[:B, :B]
                    )
                xT = xpool.tile([128, 4 * B], bf16, name=f"xT{d}", tag=f"xT{d}")
                nc.vector.tensor_copy(xT[:], pt[:])

                c = COUNTS[d]
                nm = (c + 127) // 128
                pp = ps_p.tile([128, nm * B], f32, name=f"pp{d}", tag="pp")
                for m in range(nm):
                    ms = min(128, c - m * 128)
                    for k in range(4):
                        nc.tensor.matmul(
                            pp[:ms, m * B : m * B + B],
                            lhsT=wpt_sb[
                                :, k * L + LABEL_OFF[d] + m * 128 : k * L
                                + LABEL_OFF[d] + m * 128 + ms
                            ],
                            rhs=xT[:, k * B : (k + 1) * B],
                            start=(k == 0),
                            stop=(k == 3),
                        )
                    # drain this m-chunk to DRAM while the next one multiplies
                    po = spool.tile([128, B], f32, name=f"po{d}_{m}", tag="po")
                    nc.vector.tensor_copy(po[:ms, :], pp[:ms, m * B : m * B + B])
                    row0 = LABEL_OFF[d] + m * 128
                    # mid-stream drains ride the SWDGE queue so they never
                    # perturb the wg ring cadence; only the final depth's
                    # drain takes the lower-latency HWDGE path
                    eng = nc.sync if final else nc.gpsimd
                    eng.dma_start(predT[row0 : row0 + ms, :], po[:ms, :])

            # ---- main matmuls: partial x_d = g_d @ W1_d.T, all 3 depths
            # back-to-back so the PE instruction stream has no mid-stream
            # dependencies on other engines (PE executes in order) ----
            prev = None  # (depth, psum) awaiting tail emission
            ring_i = 0
            for d in PROC:
                nch = KCH[d]
                ps = ps_x.tile([B, H], f32, name=f"psx{d}", tag="psx")
                g0 = 0
                for gi, gl in enumerate(GROUPS[d]):
                    c0 = CHUNK_OFF[d] + g0
                    # alternate the two HWDGE rings so the SDMA engines always
                    # have the next group's descriptors queued; the first
                    # three groups all ride sync because the scalar ring's
                    # FIRST transfer carries several us of extra cold-start
                    # latency that would starve the warming PE
                    ring = nc.sync if (ring_i < 3 or ring_i % 2 == 1) else nc.scalar
                    ring_i += 1
                    if d == PROC[1] and gi == 1:
                        ring.dma_start(wpt_sb[:], wpt[:])
                    wt = wpool.tile([128, gl * CHB], u8, name="wt", tag="w")
                    ring.dma_start(wt[:], wg[:, c0 * CHB : (c0 + gl) * CHB])
                    for j in range(gl):
                        nc.tensor.matmul(
                            ps[:],
                            lhsT=wt[:, j * CHB + WB : (j + 1) * CHB].bitcast(e3m4),
                            rhs=wt[:, j * CHB : j * CHB + WB].bitcast(e3m4),
                            start=(g0 + j == 0),
                            stop=(g0 + j == nch - 1),
                        )
                    g0 += gl
                    if gi == 1 and prev is not None:
                        emit_tail(*prev)
                        prev = None
                prev = (d, ps)

            emit_tail(*prev, final=True)

    nc.finalize()
    return nc


def _prep_inputs(inputs):
    emb = np.asarray(inputs["label_aware_embedding"])
    W1s = [np.asarray(inputs[f"W1_{i + 1}"]) for i in range(3)]
    Wps = [np.asarray(inputs[f"Wp_{i + 1}"]) for i in range(3)]

    wg_all = np.empty((N_CORES, 128, NCH * CHB), np.uint8)
    wgv = wg_all.reshape(N_CORES, 128, NCH, CHB)
    for d in range(3):
        ch = KCH[d]
        off = CHUNK_OFF[d]
        W1T = np.ascontiguousarray(W1s[d].T)  # [c*H, 512]
        W8 = np.clip(W1T * WSCALE, -15.5, 15.5).astype(E3M4)
        wgv[:, :, off : off + ch, :WB] = (
            W8.view(np.uint8).reshape(N_CORES, ch, 128, WB).transpose(0, 2, 1, 3)
        )
        ge = emb[:, IDX[d], :]  # [B, c, H]
        GT = np.ascontiguousarray(ge.transpose(1, 2, 0).reshape(-1, B))  # [c*H, 64]
        G8 = np.clip(GT * GSCALE, -15.5, 15.5).astype(E3M4)
        wgv[:, :, off : off + ch, WB:] = (
            G8.view(np.uint8).reshape(N_CORES, ch, 128, GB).transpose(0, 2, 1, 3)
        )

    # predictor weights absorb the 1/(WSCALE*GSCALE) dequant factor
    WPT = (np.concatenate([Wp.T for Wp in Wps], axis=1) / (WSCALE * GSCALE)).astype(
        BF16
    )
    wpt_pack = np.ascontiguousarray(
        WPT.reshape(4, 128, L).transpose(1, 0, 2).reshape(128, 4 * L)
    )

    ident = np.eye(128, dtype=BF16)

    in_maps = []
    for c in range(N_CORES):
        in_maps.append(
            {
                "wg": wg_all[c],
                "wpt": wpt_pack,
                "ident": ident,
            }
        )
    return in_maps


LAST_RESULTS = None


def kernel(**inputs):
    global LAST_RESULTS
    if "nc" not in _CACHE:
        _CACHE["nc"] = _build_module()
    nc = _CACHE["nc"]
    in_maps = _prep_inputs(inputs)
    try:
        res = bass_utils.run_bass_kernel_spmd(
            nc, in_maps, core_ids=list(range(N_CORES))
        )
    except Exception:
        # transient NRT device errors have been observed; retry once
        res = bass_utils.run_bass_kernel_spmd(
            nc, in_maps, core_ids=list(range(N_CORES))
        )
    LAST_RESULTS = res

    # unshard: contraction was sharded, so the full predictor output is the
    # sum of the per-core partials; add the bias once at the end.
    total = np.zeros((L, B), np.float64)
    for c in range(N_CORES):
        total += res.results[c]["predT"]
    bias = np.concatenate([np.asarray(inputs[f"bp_{i + 1}"]) for i in range(3)])
    total += bias.astype(np.float64)[:, None]
    out = np.empty((B, L), np.float32)
    out[:, ORDER] = total.T.astype(np.float32)
    return out


# revision 25
# speedup vs baseline: 1.0426x; 1.0426x over previous
"""Trainium2 Bass kernel for nn_Decoder (per-depth label classifier).

Math (per depth d with c_d labels, COUNTS=[16,128,512]):
    g_d = label_aware_embedding[:, idx_d, :].reshape(B, c_d*H)
    x_d = g_d @ W1_d.T                     # [B, H]
    logits_d = x_d @ Wp_d.T + bp_d         # [B, c_d]
    pred[:, idx_d] = logits_d

Sharding: the W1_d contraction dim (c_d*H) is split across 8 cores
(each core gets c_d/8 labels' worth of W1 columns plus the matching
gathered-embedding slice) and each core computes a partial x_d.
Because the predictor is linear in x, the cross-core reduction commutes
past it:  pred = (sum_i x_i) @ Wp.T = sum_i (x_i @ Wp.T).  So each core
runs the (tiny) predictor on its own partial x and the host unshard step
sums the 8 partial outputs and adds the bias once — no on-device
collective at all.

The kernel is DMA-bound on the W1 stream, so both W1 (x64) and the
gathered embedding (x2) are stored in HBM as fp8 e3m4 (Trainium's
4-mantissa-bit fp8), nearly halving HBM traffic vs bf16.  The x128
combined scale is folded into the host-packed predictor weights, so the
device tail is scale-free.  Measured rel err ~1.6e-2 vs the 2e-2
tolerance; inputs and HW accumulation are deterministic, so this margin
is stable run to run.

Device layout notes:
  - host pre-transposes so the contraction dim is the partition dim and
    every DMA reads a per-partition-contiguous span; W and g bytes are
    packed per 128-row K-chunk into one uint8 stream (576 B per chunk
    per partition: 512 B W.T e3m4, 64 B g.T e3m4) and the matmul APs
    bitcast the two sub-ranges back to e3m4.
  - main matmul: lhsT = gT chunk [128,64] (stationary), rhs = W chunk
    [128,512] (moving) -> psum [64,512] f32 per depth.
  - depths are processed largest-first (512, 128, 16 labels) so the big
    depth's tail (transpose + predictor + drain) hides inside the next
    depth's matmul stream; only the 16-label depth's tiny tail runs
    after the last main matmul.
  - the predictor needs x.T; partial x is cast to bf16 per 128-column
    block (so the PE transposes start after the first block, not the
    full cast) and transposed on the PE via identity matmuls.
"""

import sys

sys.path.insert(0, "/opt/trn_rl_repo")

import numpy as np
import ml_dtypes

import concourse.bass as bass
import concourse.bacc as bacc
import concourse.tile as tile
import concourse.mybir as mybir
from concourse import bass_utils

# bass_utils' trace path (taken when BASS_TRACE is set in the environment)
# imports antenv.axon_hooks, which this image's antenv package lacks.  Provide
# it: wire the real NTFF hook from trn_agent_boot when available, else a stub
# that degrades to an untraced run.  Also make the artifact upload a no-op
# (no bucket access here).
try:
    from antenv import axon_hooks as _axon_hooks  # noqa: F401
except ImportError:
    import types as _types

    def _make_hook():
        try:
            import trn_agent_boot.trn_boot as _tb

            return _tb._ntff_profile_via_ctypes("/opt/axon/libaxon_pjrt.so")
        except Exception:
            return None

    _hook = _make_hook()
    _mod = _types.ModuleType("antenv.axon_hooks")
    _mod.get_axon_ntff_profile_hook = lambda: _hook
    _mod.set_axon_ntff_profile_hook = lambda h: None
    sys.modules["antenv.axon_hooks"] = _mod
    bass_utils.upload_artifacts = lambda tmpdir: tmpdir

BF16 = np.dtype(ml_dtypes.bfloat16)
E3M4 = np.dtype(ml_dtypes.float8_e3m4)
WSCALE = 64.0  # W1 stored as e3m4 * WSCALE
GSCALE = 2.0  # gathered embedding stored as e3m4 * GSCALE

N_CORES = 8
H = 512
B = 64
COUNTS = [16, 128, 512]
L = sum(COUNTS)  # 656

# Fixed label->depth assignment (identical to the reference's module-level rng)
_depths = np.random.default_rng(0).permutation(np.repeat(np.arange(1, 4), COUNTS))
IDX = [np.where(_depths == d)[0] for d in (1, 2, 3)]
ORDER = np.concatenate(IDX)

PER_CORE = [c // N_CORES for c in COUNTS]  # labels per core per depth: [2, 16, 64]
KCH = [n * H // 128 for n in PER_CORE]  # K-chunks per depth per core: [8, 64, 256]
NCH = sum(KCH)  # 328

# bytes per K-chunk per partition: 512 B of W.T then 64 B of g.T (both e3m4)
WB = 512
GB = B
CHB = WB + GB  # 576

# processing order: largest depth first so its tail overlaps later streams
PROC = [2, 1, 0]
# DMA group sizes (in K-chunks) per processing position; small leading
# groups so the PE starts working as soon as possible, larger groups in
# steady state to cut per-transfer issue overhead.
GROUPS = {
    2: [8] * 6 + [16] * 13,  # 256 chunks
    1: [16] * 4,  # 64
    0: [4, 2, 2],  # 8
}
# chunk offset of each depth within the packed stream (processing order)
CHUNK_OFF = {2: 0, 1: KCH[2], 0: KCH[2] + KCH[1]}

LABEL_OFF = [0, COUNTS[0], COUNTS[0] + COUNTS[1]]  # predT row offset per depth

_CACHE = {}


def _build_module():
    f32 = mybir.dt.float32
    bf16 = mybir.dt.bfloat16
    e3m4 = mybir.dt.float8e3
    u8 = mybir.dt.uint8

    nc = bacc.Bacc("TRN2", target_bir_lowering=False, debug=False, num_devices=N_CORES)

    wg = nc.dram_tensor("wg", [128, NCH * CHB], u8, kind="ExternalInput").ap()
    wpt = nc.dram_tensor("wpt", [128, 4 * L], bf16, kind="ExternalInput").ap()
    ident = nc.dram_tensor("ident", [128, 128], bf16, kind="ExternalInput").ap()
    predT = nc.dram_tensor("predT", [L, B], f32, kind="ExternalOutput").ap()

    with tile.TileContext(nc) as tc:
        with (
            tc.tile_pool(name="wpool", bufs=12) as wpool,
            tc.tile_pool(name="consts", bufs=1) as consts,
            tc.tile_pool(name="xpool", bufs=1) as xpool,
            tc.tile_pool(name="spool", bufs=6) as spool,
            tc.tile_pool(name="ps_x", bufs=3, space="PSUM") as ps_x,
            tc.tile_pool(name="ps_t", bufs=2, space="PSUM") as ps_t,
            tc.tile_pool(name="ps_p", bufs=2, space="PSUM") as ps_p,
        ):
            # the predictor-weight load (672 KB) is deferred onto a wg HWDGE
            # ring mid-stream (below): its transfer then lands inside the
            # DMA slack opened up by the first depth's tail PE work, instead
            # of delaying the latency-critical leading wg groups.  The tiny
            # identity goes on gpsimd (SWDGE) right away.
            wpt_sb = consts.tile([128, 4 * L], bf16)
            id_sb = consts.tile([128, 128], bf16)
            nc.gpsimd.dma_start(id_sb[:], ident[:])

            # depth-d tail: cast partial x to bf16 per 128-col block,
            # transpose on the PE, then the partial predictor
            # logits_d.T = Wp_d @ x_d.T, drained to DRAM per 128-label
            # m-chunk.  For all but the last processed depth this is
            # emitted one group into the next depth's matmul stream, so
            # its PE work hides in DMA slack there.
            def emit_tail(d, ps, final=False):
                # the casts serialize regardless of engine (single PSUM-bank
                # read port), so keep them all on DVE — cross-engine hops
                # only add semaphore latency to the chain
                xbk = []
                for k in range(4):
                    xb = xpool.tile([B, 128], bf16, name=f"xb{d}_{k}", tag=f"xb{d}_{k}")
                    nc.vector.tensor_copy(xb[:], ps[:, k * 128 : (k + 1) * 128])
                    xbk.append(xb)
                pt = ps_t.tile([128, 4 * B], bf16, name=f"pt{d}", tag="pt")
                for k in range(4):
                    nc.tensor.transpose(
                        pt[:, k * B : (k + 1) * B], xbk[k][:], id_sb[:B, :B]
                    )
                xT = xpool.tile([128, 4 * B], bf16, name=f"xT{d}", tag=f"xT{d}")
                nc.vector.tensor_copy(xT[:], pt[:])

                c = COUNTS[d]
                nm = (c + 127) // 128
                pp = ps_p.tile([128, nm * B], f32, name=f"pp{d}", tag="pp")
                for m in range(nm):
                    ms = min(128, c - m * 128)
                    for k in range(4):
                        nc.tensor.matmul(
                            pp[:ms, m * B : m * B + B],
                            lhsT=wpt_sb[
                                :, k * L + LABEL_OFF[d] + m * 128 : k * L
                                + LABEL_OFF[d] + m * 128 + ms
                            ],
                            rhs=xT[:, k * B : (k + 1) * B],
                            start=(k == 0),
                            stop=(k == 3),
                        )
                    # drain this m-chunk to DRAM while the next one multiplies
                    po = spool.tile([128, B], f32, name=f"po{d}_{m}", tag="po")
                    nc.vector.tensor_copy(po[:ms, :], pp[:ms, m * B : m * B + B])
                    row0 = LABEL_OFF[d] + m * 128
                    # mid-stream drains ride the SWDGE queue so they never
                    # perturb the wg ring cadence; only the final depth's
                    # drain takes the lower-latency HWDGE path
                    eng = nc.sync if final else nc.gpsimd
                    eng.dma_start(predT[row0 : row0 + ms, :], po[:ms, :])

            # ---- main matmuls: partial x_d = g_d @ W1_d.T, all 3 depths
            # back-to-back so the PE instruction stream has no mid-stream
            # dependencies on other engines (PE executes in order) ----
            prev = None  # (depth, psum) awaiting tail emission
            ring_i = 0
            for d in PROC:
                nch = KCH[d]
                ps = ps_x.tile([B, H], f32, name=f"psx{d}", tag="psx")
                g0 = 0
                for gi, gl in enumerate(GROUPS[d]):
                    c0 = CHUNK_OFF[d] + g0
                    # alternate the two HWDGE rings so the SDMA engines always
                    # have the next group's descriptors queued
                    ring = nc.sync if ring_i % 2 == 0 else nc.scalar
                    ring_i += 1
                    if d == PROC[1] and gi == 1:
                        ring.dma_start(wpt_sb[:], wpt[:])
                    wt = wpool.tile([128, gl * CHB], u8, name="wt", tag="w")
                    ring.dma_start(wt[:], wg[:, c0 * CHB : (c0 + gl) * CHB])
                    for j in range(gl):
                        nc.tensor.matmul(
                            ps[:],
                            lhsT=wt[:, j * CHB + WB : (j + 1) * CHB].bitcast(e3m4),
                            rhs=wt[:, j * CHB : j * CHB + WB].bitcast(e3m4),
                            start=(g0 + j == 0),
                            stop=(g0 + j == nch - 1),
                        )
                    g0 += gl
                    if gi == 1 and prev is not None:
                        emit_tail(*prev)
                        prev = None
                prev = (d, ps)

            emit_tail(*prev, final=True)

    nc.finalize()
    return nc


def _prep_inputs(inputs):
    emb = np.asarray(inputs["label_aware_embedding"])
    W1s = [np.asarray(inputs[f"W1_{i + 1}"]) for i in range(3)]
    Wps = [np.asarray(inputs[f"Wp_{i + 1}"]) for i in range(3)]

    wg_all = np.empty((N_CORES, 128, NCH * CHB), np.uint8)
    wgv = wg_all.reshape(N_CORES, 128, NCH, CHB)
    for d in range(3):
        ch = KCH[d]
        off = CHUNK_OFF[d]
        W1T = np.ascontiguousarray(W1s[d].T)  # [c*H, 512]
        W8 = np.clip(W1T * WSCALE, -15.5, 15.5).astype(E3M4)
        wgv[:, :, off : off + ch, :WB] = (
            W8.view(np.uint8).reshape(N_CORES, ch, 128, WB).transpose(0, 2, 1, 3)
        )
        ge = emb[:, IDX[d], :]  # [B, c, H]
        GT = np.ascontiguousarray(ge.transpose(1, 2, 0).reshape(-1, B))  # [c*H, 64]
        G8 = np.clip(GT * GSCALE, -15.5, 15.5).astype(E3M4)
        wgv[:, :, off : off + ch, WB:] = (
            G8.view(np.uint8).reshape(N_CORES, ch, 128, GB).transpose(0, 2, 1, 3)
        )

    # predictor weights absorb the 1/(WSCALE*GSCALE) dequant factor
    WPT = (np.concatenate([Wp.T for Wp in Wps], axis=1) / (WSCALE * GSCALE)).astype(
        BF16
    )
    wpt_pack = np.ascontiguousarray(
        WPT.reshape(4, 128, L).transpose(1, 0, 2).reshape(128, 4 * L)
    )

    ident = np.eye(128, dtype=BF16)

    in_maps = []
    for c in range(N_CORES):
        in_maps.append(
            {
                "wg": wg_all[c],
                "wpt": wpt_pack,
                "ident": ident,
            }
        )
    return in_maps


LAST_RESULTS = None


def kernel(**inputs):
    global LAST_RESULTS
    if "nc" not in _CACHE:
        _CACHE["nc"] = _build_module()
    nc = _CACHE["nc"]
    in_maps = _prep_inputs(inputs)
    try:
        res = bass_utils.run_bass_kernel_spmd(
            nc, in_maps, core_ids=list(range(N_CORES))
        )
    except Exception:
        # transient NRT device errors have been observed; retry once
        res = bass_utils.run_bass_kernel_spmd(
            nc, in_maps, core_ids=list(range(N_CORES))
        )
    LAST_RESULTS = res

    # unshard: contraction was sharded, so the full predictor output is the
    # sum of the per-core partials; add the bias once at the end.
    total = np.zeros((L, B), np.float64)
    for c in range(N_CORES):
        total += res.results[c]["predT"]
    bias = np.concatenate([np.asarray(inputs[f"bp_{i + 1}"]) for i in range(3)])
    total += bias.astype(np.float64)[:, None]
    out = np.empty((B, L), np.float32)
    out[:, ORDER] = total.T.astype(np.float32)
    return out
